# revision 2
# baseline (speedup 1.0000x reference)
"""Trainium2 Bass kernel for nn_LMEncoder segment-reduce.

Math (from the reference):
  x = mean over the 4 layers of hidden_last4          [B, S, H]
  out[b,t] = sum_{k=1..span[b,t]} x[b, t+k]   for 1 <= t < mask_len-1, else 0

v6 "uniform pair-packed" design:
  - hidden_last4 uploaded as bf16 (host cast). Band weights are {0,0.25},
    exact in bf16.
  - Every 128-token tile: 4 chunk DMAs [128 = 2 layers x 64 tokens, 768];
    one DVE bf16 add per token half merges the layer pairs (partitions
    aligned); PE contracts (half-sum, token) with the band weights
    replicated over the 2 groups: 3 PSUM passes per half (2 chunks +
    3-token spill), 2 for sequence-final tiles.
  - Chunk loads are placed by a greedy scheduler: the two chunks of each
    half-sum arrive on the two least-busy DMA queues in the order DVE
    consumes them; W pieces are injected just-in-time; Activation joins
    loading early and switches to PSUM copies later.
  - Both PSUM halves live in one 2-bank PSUM tile; a single strided copy
    (Act for early tiles, DVE for late ones) writes the big per-sequence
    out tile; four flat half-sequence stores go to DRAM.

Sharding: batch dim 16 -> 2 sequences per core x 8 cores.
"""

import os
import sys

import numpy as np

for _p in ("/opt/trn_rl_repo", "/root/.axon_site/_ro/trn_rl_repo"):
    if os.path.isdir(_p) and _p not in sys.path:
        sys.path.insert(0, _p)

from concourse import bacc, bass, mybir, tile  # noqa: E402
from concourse.bass_utils import run_bass_kernel_spmd  # noqa: E402

from ml_dtypes import bfloat16 as np_bf16  # noqa: E402

B, S, H = 16, 512, 768
P = 128
MT = S // P
NT = 2 * MT
NCORES = 8
BL = B // NCORES
NF = 384
PSF = 1024            # 2-bank psum tile; halves at offsets 0 and 512

# copies: Act for the first tiles (DVE still summing), DVE for the tail
_COPY = {i: c for i, c in enumerate("aaaaavva")}
N_WARM = 36
ACT_CK = 6
# four half-sequence stores (b, half) -> queue (issued as copies complete)
_STOREQ = {(0, 0): "sync", (0, 1): "gpsimd", (1, 0): "sync", (1, 1): "gpsimd"}

_CK = 592.0           # chunk transfer ns
_WP = 592.0           # W piece ns (approx per-piece)
_ACT_T0 = 1483.0      # Act queue available after the act-table load

_CACHE = {}


def _schedule_loads():
    """Greedy queue schedule. Returns ordered job list per queue.
    Jobs: ("w", piece), ("ck", i, h, g)."""
    qt = {"sync": 200.0, "gpsimd": 200.0, "scalar": _ACT_T0}
    qjobs = {"sync": [], "gpsimd": [], "scalar": []}
    act_ck = 0
    # W pieces: tiles 0-1 (piece 0) first on sync; rest mid-stream
    qjobs["sync"].append(("w", 0))
    qt["sync"] += _WP
    wqueued = 1
    for k, (i, h) in enumerate([(i, h) for i in range(NT) for h in range(2)]):
        # inject next W piece when half the pairs of its span approach
        if wqueued < 4 and k >= 3 * wqueued + 1:
            q = min(qt, key=lambda x: qt[x])
            qjobs[q].append(("w", wqueued))
            qt[q] += _WP
            wqueued += 1
        avail = [q for q in qt if q != "scalar" or act_ck < ACT_CK]
        qs = sorted(avail, key=lambda x: qt[x])[:2]
        if len(qs) < 2:
            qs = sorted(qt, key=lambda x: qt[x])[:2]
        for g, q in enumerate(qs):
            qjobs[q].append(("ck", i, h, g))
            qt[q] += _CK
            if q == "scalar":
                act_ck += 1
    while wqueued < 4:
        q = min(qt, key=lambda x: qt[x])
        qjobs[q].append(("w", wqueued))
        qt[q] += _WP
        wqueued += 1
    return qjobs


def _build_nc():
    # W columns: per tile 2 chunk blocks; +spill block if tile has successor
    colA = {}
    cols = 0
    for i in range(NT):
        for c in range(2):
            colA[("wc", i, c)] = cols
            cols += P
        if i % MT != MT - 1:
            colA[("wsp", i)] = cols
            cols += P
    # 4 W pieces, split at tile boundaries [0..2), [2..4), [4..6), [6..8)
    bounds = []
    for pc in range(4):
        lo_tile = 2 * pc
        lo = colA[("wc", lo_tile, 0)]
        bounds.append(lo)
    bounds.append(cols)

    nc = bacc.Bacc(None, target_bir_lowering=False)
    hb = nc.dram_tensor("hb", [4, BL, S, H], mybir.dt.bfloat16, kind="ExternalInput")
    wa = nc.dram_tensor("wa", [P, cols], mybir.dt.bfloat16, kind="ExternalInput")
    o = nc.dram_tensor("o", [BL, 2, P * 2 * H], mybir.dt.float32, kind="ExternalOutput")

    qeng = {"sync": nc.sync, "scalar": nc.scalar, "gpsimd": nc.gpsimd}

    with tile.TileContext(nc) as tc:
        with tc.tile_pool(name="w", bufs=1) as wpool, \
             tc.tile_pool(name="xc", bufs=32) as xcpool, \
             tc.tile_pool(name="xs", bufs=10) as xspool, \
             tc.tile_pool(name="out", bufs=4) as opool, \
             tc.tile_pool(name="ps", bufs=3, space="PSUM") as pspool, \
             tc.tile_pool(name="pd", bufs=1, space="PSUM") as pdpool:
            wat = wpool.tile([P, cols], mybir.dt.bfloat16, name="wat")
            if N_WARM:
                dummy = wpool.tile([P, P], mybir.dt.bfloat16, name="dummy")
                nc.vector.memzero(dummy[:])
                psd = pdpool.tile([P, 64], mybir.dt.float32, name="psd",
                                  padded_shape=[P, 512])
                for _k in range(N_WARM):
                    nc.tensor.matmul(psd[:], dummy[:, 0:P], dummy[:, 0:64],
                                     start=True, stop=True,
                                     skip_group_check=True)
            cts = {}

            qjobs = _schedule_loads()
            maxlen = max(len(v) for v in qjobs.values())
            for r in range(maxlen):
                for q in ("sync", "gpsimd", "scalar"):
                    if r >= len(qjobs[q]):
                        continue
                    job = qjobs[q][r]
                    if job[0] == "w":
                        lo, hi = bounds[job[1]], bounds[job[1] + 1]
                        qeng[q].dma_start(wat[:, lo:hi], wa[:, lo:hi])
                    else:
                        _, i, h, g = job
                        b, m = divmod(i, MT)
                        ct = xcpool.tile([P, H], mybir.dt.bfloat16, tag="xc",
                                         name="ct")
                        s0 = m * P + 64 * h
                        qeng[q].dma_start(ct[:],
                                          hb[2 * g:2 * g + 2, b, s0:s0 + 64, :])
                        cts[(i, h, g)] = ct

            # ---- DVE pair adds ----
            sms = {}
            for i in range(NT):
                for h in range(2):
                    sm = xspool.tile([P, H], mybir.dt.bfloat16, tag="xs",
                                     name="sm")
                    nc.vector.tensor_add(sm[:], cts[(i, h, 0)][:],
                                         cts[(i, h, 1)][:])
                    sms[(i, h)] = sm

            # ---- PE + copies + stores ----
            ots = {}
            for b in range(BL):
                for half in range(2):
                    ots[(b, half)] = opool.tile([P, 2 * H], mybir.dt.float32,
                                                tag="o", name="ot")
            for i in range(NT):
                b, m = divmod(i, MT)
                has_next = (m != MT - 1)
                ps = pspool.tile([P, PSF], mybir.dt.float32, tag="ps",
                                 name="ps")
                for n in range(2):
                    pslice = slice(n * 512, n * 512 + NF)
                    nf = slice(n * NF, (n + 1) * NF)
                    for c in range(2):
                        c0 = colA[("wc", i, c)]
                        nc.tensor.matmul(ps[:, pslice], wat[:, c0:c0 + P],
                                         sms[(i, c)][:, nf],
                                         start=(c == 0),
                                         stop=(not has_next and c == 1))
                    if has_next:
                        c1 = colA[("wsp", i)]
                        nc.tensor.matmul(ps[:, pslice], wat[:, c1:c1 + P],
                                         sms[(i + 1, 0)][:, nf],
                                         start=False, stop=True)
                src = ps[:, 0:PSF].rearrange("p (k f) -> p k f", k=2)[:, :, 0:NF]
                dst = ots[(b, m // 2)][:, (m % 2) * H:(m % 2 + 1) * H]
                if _COPY[i] == "v":
                    nc.vector.tensor_copy(dst, src)
                else:
                    nc.scalar.copy(dst, src)
                if m % 2 == 1:
                    half = m // 2
                    qeng[_STOREQ[(b, half)]].dma_start(
                        o[b, half, :], ots[(b, half)][:].flatten())
    nc.finalize()
    return nc, colA, cols


def _coeffs(lm_spans, masks):
    t = np.arange(S)
    mask_len = masks.astype(np.int64).sum(axis=1)
    valid = (t[None, :] >= 1) & (t[None, :] < (mask_len[:, None] - 1))
    span_eff = np.minimum(lm_spans.astype(np.int64), (S - 1 - t)[None, :])
    c = np.zeros((3, B, S), np.float32)
    for d in (1, 2, 3):
        c[d - 1] = 0.25 * (valid & (span_eff >= d)).astype(np.float32)
    return c


def _build_w_full(lm_spans, masks):
    c = _coeffs(lm_spans, masks)
    t = np.arange(S)
    wfull = np.zeros((B, S + 3, S), np.float32)
    for d in (1, 2, 3):
        wfull[:, t + d, t] = c[d - 1][:, t]
    return wfull[:, :S, :]


def _make_inmaps(hidden_last4, lm_spans, masks, colA, cols):
    wfull = _build_w_full(np.asarray(lm_spans), np.asarray(masks))
    hidden_last4 = np.asarray(hidden_last4)
    in_maps = []
    for ci in range(NCORES):
        bs = slice(BL * ci, BL * (ci + 1))
        wf = wfull[bs]
        wa = np.zeros((P, cols), np.float32)
        for key, c0 in colA.items():
            if key[0] == "wc":
                _, i, c = key
                b, m = divmod(i, MT)
                rows = wf[b, m * P + 64 * c:m * P + 64 * (c + 1),
                          m * P:(m + 1) * P]
                wa[:, c0:c0 + P] = np.tile(rows, (2, 1))
            else:  # wsp: spill from tile i+1's first half-sum (pair layout)
                _, i = key
                b, m = divmod(i, MT)
                rows = np.zeros((P, P), np.float32)
                sp = wf[b, (m + 1) * P:(m + 1) * P + 3, m * P:(m + 1) * P]
                rows[0:3] = sp
                rows[64:67] = sp
                wa[:, c0:c0 + P] = rows
        in_maps.append({
            "hb": np.ascontiguousarray(hidden_last4[:, bs]).astype(np_bf16),
            "wa": wa.astype(np_bf16),
        })
    return in_maps


def _get_nc():
    if "nc" not in _CACHE:
        _CACHE["nc"] = _build_nc()
    return _CACHE["nc"]


def _decode_out(o_flat):
    # o[b, half, :] is [128, 2*768] flattened partition-major
    o = o_flat.reshape(BL, 2, P, 2, H)          # b, half, p, m2, h
    o = o.transpose(0, 1, 3, 2, 4)              # b, half, m2, p, h
    return o.reshape(BL, S, H)


def _run(hidden_last4, lm_spans, masks, **spmd_kwargs):
    nc, colA, cols = _get_nc()
    in_maps = _make_inmaps(hidden_last4, lm_spans, masks, colA, cols)
    res = run_bass_kernel_spmd(nc, in_maps, core_ids=list(range(NCORES)), **spmd_kwargs)
    out = np.concatenate([_decode_out(r["o"]) for r in res.results], axis=0)
    return out, res


def kernel(hidden_last4, lm_spans, masks):
    out, _ = _run(hidden_last4, lm_spans, masks)
    return out
